# revision 30
# baseline (speedup 1.0000x reference)
"""Trainium2 Bass kernel for nn_CovarianceLayer (Toeplitz-autocorrelation form).

Math: x = inputs[:,0,:] + i*inputs[:,1,:]  (B=256 complex signals, N=1024)
      cov[b,l,m] = Re(hankel @ hankel^H)[l,m] / L  with hankel[b,i,j] = x[b,(j+i)%N]
By circularity cov[b,l,m] = r_b[|l-m|] / L where
      r_b[d] = sum_n ( xr[n]xr[n+d] + xi[n]xi[n+d] )   (indices mod N)
i.e. each [L,L] output tile is a symmetric Toeplitz matrix fully
determined by a 128-lag autocorrelation r_b, so only r_b is computed.

Per-core pipeline (32 batches/core, pure data parallel):
  1. One gpsimd casting DMA per supergroup builds a doubled fp8e4m3 copy
     of x in DRAM (row per batch: [x0 x0 | x1 x1]; the duplication
     realizes the circular wrap). Separate DRAM tiles per supergroup
     keep the dependency tracking fine-grained.
  2. Per 8-batch supergroup, 2 HWDGE DMAs build a packed Hankel tile
     H[16c+p, j*W+u] = x_c[b_j, p+u] (overlapping-window access
     pattern; comps fp8-packed on partition halves).
  3. 32 DoubleRow fp8 matmuls per batch (each contracts K=32 offsets x
     comps times 2 k-tiles) accumulate r_b into one psum column; the
     moving operand is a single column of the same Hankel tile.
  4. Per compute group: DVE drains psum -> SBUF, then ONE f32r matmul
     against a constant palindrome matrix (spal[d,k] = [d==|k-127|]/L)
     applies transpose + mirror + 1/L in one PE op, and a single DVE
     copy stages the palindrome rows s_b[k] = r_b[|k-127|] in SBUF.
  5. One strided DMA per group expands the Toeplitz tiles straight from
     SBUF into the output: out[b,l,m] = s_b[127-l+m] (contiguous 512B
     runs on both sides, ~360GB/s on the DMA engines).
Group sizes taper so the first expansion DMA starts early and the last
group's drain->expand tail is short; expansion DMAs alternate SP/Act
queues and overlap later groups' matmuls.
"""

import numpy as np

import concourse.bacc as bacc
import concourse.mybir as mybir
import concourse.tile as tile
from concourse.bass_types import AP
from concourse.bass_utils import run_bass_kernel_spmd

B, L, N = 256, 128, 1024
NCORES = 8
BPC = B // NCORES  # 32 batches per core

P = 16  # n-offsets per chunk
K = 2 * P  # matmul contraction width
T = N // P  # 64 chunks per batch
W = N - P + 128  # 1136
CROW = 2 * N  # doubled per-comp row in xdup
ROW = 2 * CROW  # 4096 elems per batch

_CACHE = {}
LAST_RESULT = None

SGROUPS = [8, 8, 8, 8]  # hankel supergroup sizes
GROUPS = [6, 6, 6, 6, 4, 2, 2]  # compute/expansion group sizes
CAST_SPLIT = 0  # batches cast in the first (fast) casting DMA; 0 = single


def build_nc(sgroups=None, groups=None, cast_split=CAST_SPLIT, act_copy=False,
             warmup=False, ident_input=True, last_sp=0, sp_only=False,
             pal_mm="f32r", cgroups=None, fin_at=None, pbufs=(4, 4),
             spal_bf16=False, spal_late=True, spal_split=False):
    f8 = mybir.dt.float8e4
    f32 = mybir.dt.float32
    sgroups = list(SGROUPS if sgroups is None else sgroups)
    groups = list(GROUPS if groups is None else groups)
    assert sum(sgroups) == BPC and sum(groups) == BPC
    sstart = [sum(sgroups[:i]) for i in range(len(sgroups))]
    gstart = [sum(groups[:i]) for i in range(len(groups))]
    ng = len(groups)
    # map batch -> supergroup index
    b2s = []
    for s, sz in enumerate(sgroups):
        b2s += [s] * sz

    nc = bacc.Bacc(
        "TRN2", target_bir_lowering=False, debug=False, num_devices=NCORES
    )
    inp = nc.dram_tensor("inp", [BPC, 2, N], f32, kind="ExternalInput")
    pal_dt = {None: None, "f32": f32, "f32r": mybir.dt.float32r}[pal_mm]
    identd = (
        nc.dram_tensor("identd", [128, 128], f32, kind="ExternalInput")
        if ident_input and not pal_mm
        else None
    )
    spal_dt = {
        "bf16": mybir.dt.bfloat16,
        "f8": mybir.dt.float8e4,
        False: pal_dt,
        True: mybir.dt.bfloat16,
    }[spal_bf16]
    spald = (
        nc.dram_tensor("spald", [128, 256], spal_dt, kind="ExternalInput")
        if pal_mm
        else None
    )
    out = nc.dram_tensor("out", [BPC, L, L], f32, kind="ExternalOutput")

    with tile.TileContext(nc) as tc:
        with (
            tc.tile_pool(name="const", bufs=1) as cpool,
            tc.tile_pool(name="dram", bufs=1, space="DRAM") as dpool,
            tc.tile_pool(name="hank", bufs=len(sgroups)) as hpool,
            tc.tile_pool(name="spal", bufs=ng) as spool,
            tc.tile_pool(name="rr", bufs=ng) as rpool,
            tc.tile_pool(name="psum", bufs=pbufs[0], space="PSUM") as ppool,
            tc.tile_pool(name="pst", bufs=pbufs[1], space="PSUM") as tpool,
        ):
            # --- doubled fp8 signal in DRAM via per-supergroup casting
            # DMAs, emitted FIRST. Each supergroup gets its own DRAM tile so
            # its hankel DMAs depend only on its own cast (tile-granular
            # dependency tracking). ---
            flat = inp[:].rearrange("b c n -> (b c) n")
            cgs = list(cgroups) if cgroups else list(sgroups)
            assert sum(cgs) == BPC
            cstart = [sum(cgs[:i]) for i in range(len(cgs))]
            xdups = []  # one per cast group
            for si, ssz in enumerate(cgs):
                xd = dpool.tile([2 * ssz, CROW], f8)
                xdups.append(xd)
                r0 = 2 * cstart[si]
                src0 = AP(
                    tensor=flat.tensor,
                    offset=flat.offset + r0 * N,
                    ap=[[N, 2 * ssz], [0, 2], [1, N]],
                )
                dst0 = AP(
                    tensor=xd.tensor,
                    offset=xd.offset,
                    ap=[[CROW, 2 * ssz], [N, 2], [1, N]],
                )
                nc.gpsimd.dma_start(out=dst0, in_=src0)

            # --- identity for PE transpose (needed only ~9us in) ---
            def load_spal():
                if spal_split:
                    nc.scalar.dma_start(out=spal_t[:, 0:128], in_=spald[:, 0:128])
                    nc.sync.dma_start(out=spal_t[:, 128:256], in_=spald[:, 128:256])
                else:
                    (nc.sync if sp_only else nc.scalar).dma_start(
                        out=spal_t[:], in_=spald[:]
                    )

            if pal_mm:
                spal_t = cpool.tile([128, 256], spal_dt)
                if not spal_late:
                    load_spal()
                ident = None
            else:
                ident = cpool.tile([128, 128], f32)
                if ident_input:
                    (nc.sync if sp_only else nc.scalar).dma_start(
                        out=ident[:], in_=identd[:]
                    )
                else:
                    ones = cpool.tile([128, 128], f32)
                    nc.vector.memset(ones[:], 1.0)
                    nc.gpsimd.affine_select(
                        out=ident[:],
                        in_=ones[:],
                        pattern=[[1, 128]],
                        compare_op=mybir.AluOpType.is_equal,
                        fill=0.0,
                        base=0,
                        channel_multiplier=-1,
                    )
            if warmup:
                warm = cpool.tile([1, 2], f32)
                nc.vector.memset(warm[:], 1.0)
                nc.scalar.mul(warm[:], warm[:], 1.0)

            # --- hankel tiles: H[16c+p, j*W+u] = x_c[b_j, p+u] ---
            htiles = []
            for s, ssz in enumerate(sgroups):
                ht = hpool.tile([K, ssz * W], f8)
                htiles.append(ht)
                ci = max(i for i in range(len(cgs)) if cstart[i] <= sstart[s])
                assert cstart[ci] + cgs[ci] >= sstart[s] + ssz, "sg spans casts"
                xd = xdups[ci]
                xoff = 2 * (sstart[s] - cstart[ci]) * CROW
                if ssz == 1:
                    src = AP(
                        tensor=xd.tensor,
                        offset=xd.offset + xoff,
                        ap=[[CROW, 2], [1, P], [1, W]],
                    )
                    eng = nc.sync if (sp_only or s % 2 == 0) else nc.scalar
                    eng.dma_start(out=ht[:], in_=src)
                else:
                    for c in range(2):
                        src = AP(
                            tensor=xd.tensor,
                            offset=xd.offset + xoff + c * CROW,
                            ap=[[1, P], [ROW, ssz], [1, W]],
                        )
                        eng = nc.sync if (sp_only or c == 0) else nc.scalar
                        eng.dma_start(out=ht[P * c : P * c + P, :], in_=src)

            if pal_mm and spal_late:
                load_spal()

            rgs = {}

            def finish(g):
                gb = groups[g]
                rows = spool.tile([gb, 256], f32)
                if pal_mm:
                    # one matmul applies transpose+mirror+1/L:
                    # pt2[j,k] = sum_d rg[d,j]*spal[d,k],  spal[d,k]=[d==|k-127|]/L
                    pt2 = tpool.tile([gb, 256], f32)
                    nc.tensor.matmul(pt2[:], rgs[g][:], spal_t[:])
                    nc.vector.tensor_copy(rows[:, 0:255], pt2[:, 0:255])
                else:
                    pt = tpool.tile([gb, 128], f32)
                    nc.tensor.transpose(pt[:], rgs[g][:], ident[:])
                    if act_copy is True or sp_only or (act_copy and g in act_copy):
                        nc.scalar.mul(rows[:, 127:255], pt[:], 1.0)
                    else:
                        nc.vector.tensor_copy(rows[:, 127:255], pt[:])
                    nc.vector.tensor_copy(rows[:, 0:127], pt[:, 127:0:-1])
                src2 = AP(
                    tensor=rows.tensor,
                    offset=rows.offset + 127,
                    ap=[[256, gb], [-1, 128], [1, 128]],
                )
                dst2 = AP(
                    tensor=out,
                    offset=gstart[g] * L * L,
                    ap=[[L * L, gb], [L, 128], [1, 128]],
                )
                if sp_only or g >= ng - last_sp:
                    eng = nc.sync
                else:
                    eng = nc.sync if g % 2 == 0 else nc.scalar
                eng.dma_start(out=dst2, in_=src2)

            for g in range(ng):
                gb = groups[g]
                ps = ppool.tile([128, gb], f32)
                fa = fin_at if fin_at is not None else max(1, gb // 2)
                for j in range(gb):
                    if j == min(fa, gb - 1) and g >= 1:
                        finish(g - 1)
                    b = gstart[g] + j
                    s = b2s[b]
                    ht = htiles[s]
                    col = (b - sstart[s]) * W
                    for tp in range(T // 2):
                        off = col + K * tp
                        lhsT = AP(
                            tensor=ht.tensor,
                            offset=ht.offset + off,
                            ap=[[sgroups[s] * W, K], [P, 2], [1, 128]],
                        )
                        rhs = AP(
                            tensor=ht.tensor,
                            offset=ht.offset + off,
                            ap=[[sgroups[s] * W, K], [P, 2], [1, 1]],
                        )
                        nc.tensor.matmul(
                            ps[:, j : j + 1],
                            lhsT,
                            rhs,
                            start=(tp == 0),
                            stop=(tp == T // 2 - 1),
                            perf_mode=mybir.MatmulPerfMode.DoubleRow,
                        )

                rg = rpool.tile([128, gb], pal_dt if pal_mm else f32)
                if pal_mm:
                    nc.vector.tensor_copy(rg[:], ps[:])
                else:
                    nc.vector.tensor_scalar_mul(rg[:], ps[:], 1.0 / L)
                rgs[g] = rg
            finish(ng - 1)

    nc.compile()
    return nc


def kernel(inputs: np.ndarray) -> np.ndarray:
    global LAST_RESULT
    inputs = np.ascontiguousarray(np.asarray(inputs), dtype=np.float32)
    assert inputs.shape == (B, 2, N), inputs.shape

    if "nc" not in _CACHE:
        _CACHE["nc"] = build_nc()
    nc = _CACHE["nc"]

    k = np.arange(256)
    d = np.arange(128)
    spal = (d[:, None] == np.minimum(np.abs(k[None, :] - 127), 127)).astype(
        np.float32
    ) / L
    spal[:, 255] = 0.0
    in_maps = [
        {"inp": inputs[c * BPC : (c + 1) * BPC], "spald": spal}
        for c in range(NCORES)
    ]
    res = run_bass_kernel_spmd(nc, in_maps, list(range(NCORES)), trace=False)
    LAST_RESULT = res
    outf = np.concatenate([res.results[c]["out"] for c in range(NCORES)], axis=0)
    return outf.reshape(B, L, L, 1).astype(np.float32, copy=False)


# revision 32
# speedup vs baseline: 1.0014x; 1.0014x over previous
"""Trainium2 Bass kernel for nn_CovarianceLayer (Toeplitz-autocorrelation form).

Math: x = inputs[:,0,:] + i*inputs[:,1,:]  (B=256 complex signals, N=1024)
      cov[b,l,m] = Re(hankel @ hankel^H)[l,m] / L  with hankel[b,i,j] = x[b,(j+i)%N]
By circularity cov[b,l,m] = r_b[|l-m|] / L where
      r_b[d] = sum_n ( xr[n]xr[n+d] + xi[n]xi[n+d] )   (indices mod N)
i.e. each [L,L] output tile is a symmetric Toeplitz matrix fully
determined by a 128-lag autocorrelation r_b, so only r_b is computed.

Per-core pipeline (32 batches/core, pure data parallel):
  1. One gpsimd casting DMA per supergroup builds a doubled fp8e4m3 copy
     of x in DRAM (row per batch: [x0 x0 | x1 x1]; the duplication
     realizes the circular wrap). Separate DRAM tiles per supergroup
     keep the dependency tracking fine-grained.
  2. Per 8-batch supergroup, 2 HWDGE DMAs build a packed Hankel tile
     H[16c+p, j*W+u] = x_c[b_j, p+u] (overlapping-window access
     pattern; comps fp8-packed on partition halves).
  3. 32 DoubleRow fp8 matmuls per batch (each contracts K=32 offsets x
     comps times 2 k-tiles) accumulate r_b into one psum column; the
     moving operand is a single column of the same Hankel tile.
  4. Per compute group: DVE drains psum -> SBUF, then ONE f32r matmul
     against a constant palindrome matrix (spal[d,k] = [d==|k-127|]/L)
     applies transpose + mirror + 1/L in one PE op, and a single DVE
     copy stages the palindrome rows s_b[k] = r_b[|k-127|] in SBUF.
  5. One strided DMA per group expands the Toeplitz tiles straight from
     SBUF into the output: out[b,l,m] = s_b[127-l+m] (contiguous 512B
     runs on both sides, ~360GB/s on the DMA engines).
Group sizes taper so the first expansion DMA starts early and the last
group's drain->expand tail is short; expansion DMAs alternate SP/Act
queues and overlap later groups' matmuls.
"""

import numpy as np

import concourse.bacc as bacc
import concourse.mybir as mybir
import concourse.tile as tile
from concourse.bass_types import AP
from concourse.bass_utils import run_bass_kernel_spmd

B, L, N = 256, 128, 1024
NCORES = 8
BPC = B // NCORES  # 32 batches per core

P = 16  # n-offsets per chunk
K = 2 * P  # matmul contraction width
T = N // P  # 64 chunks per batch
W = N - P + 128  # 1136
CROW = 2 * N  # doubled per-comp row in xdup
ROW = 2 * CROW  # 4096 elems per batch

_CACHE = {}
LAST_RESULT = None

SGROUPS = [8, 8, 8, 8]  # hankel supergroup sizes
GROUPS = [6, 6, 6, 6, 4, 2, 2]  # compute/expansion group sizes
CAST_SPLIT = 0  # batches cast in the first (fast) casting DMA; 0 = single


def build_nc(sgroups=None, groups=None, cast_split=CAST_SPLIT, act_copy=False,
             warmup=False, ident_input=True, last_sp=0, sp_only=False,
             pal_mm="f32r", cgroups=None, fin_at=None, pbufs=(4, 4),
             spal_bf16=False, spal_late=True, spal_split=False,
             hswap=False, spal_pool=True):
    f8 = mybir.dt.float8e4
    f32 = mybir.dt.float32
    sgroups = list(SGROUPS if sgroups is None else sgroups)
    groups = list(GROUPS if groups is None else groups)
    assert sum(sgroups) == BPC and sum(groups) == BPC
    sstart = [sum(sgroups[:i]) for i in range(len(sgroups))]
    gstart = [sum(groups[:i]) for i in range(len(groups))]
    ng = len(groups)
    # map batch -> supergroup index
    b2s = []
    for s, sz in enumerate(sgroups):
        b2s += [s] * sz

    nc = bacc.Bacc(
        "TRN2", target_bir_lowering=False, debug=False, num_devices=NCORES
    )
    inp = nc.dram_tensor("inp", [BPC, 2, N], f32, kind="ExternalInput")
    pal_dt = {None: None, "f32": f32, "f32r": mybir.dt.float32r}[pal_mm]
    identd = (
        nc.dram_tensor("identd", [128, 128], f32, kind="ExternalInput")
        if ident_input and not pal_mm
        else None
    )
    spal_dt = {
        "bf16": mybir.dt.bfloat16,
        "f8": mybir.dt.float8e4,
        False: pal_dt,
        True: mybir.dt.bfloat16,
    }[spal_bf16]
    spald = (
        nc.dram_tensor("spald", [128, 256], spal_dt, kind="ExternalInput")
        if pal_mm
        else None
    )
    out = nc.dram_tensor("out", [BPC, L, L], f32, kind="ExternalOutput")

    with tile.TileContext(nc) as tc:
        with (
            tc.tile_pool(name="const", bufs=1) as cpool,
            tc.tile_pool(name="dram", bufs=1, space="DRAM") as dpool,
            tc.tile_pool(name="hank", bufs=len(sgroups)) as hpool,
            tc.tile_pool(name="spal", bufs=ng) as spool,
            tc.tile_pool(name="rr", bufs=ng) as rpool,
            tc.tile_pool(name="psum", bufs=pbufs[0], space="PSUM") as ppool,
            tc.tile_pool(name="pst", bufs=pbufs[1], space="PSUM") as tpool,
        ):
            # --- doubled fp8 signal in DRAM via per-supergroup casting
            # DMAs, emitted FIRST. Each supergroup gets its own DRAM tile so
            # its hankel DMAs depend only on its own cast (tile-granular
            # dependency tracking). ---
            flat = inp[:].rearrange("b c n -> (b c) n")
            cgs = list(cgroups) if cgroups else list(sgroups)
            assert sum(cgs) == BPC
            cstart = [sum(cgs[:i]) for i in range(len(cgs))]
            xdups = []  # one per cast group
            for si, ssz in enumerate(cgs):
                xd = dpool.tile([2 * ssz, CROW], f8)
                xdups.append(xd)
                r0 = 2 * cstart[si]
                src0 = AP(
                    tensor=flat.tensor,
                    offset=flat.offset + r0 * N,
                    ap=[[N, 2 * ssz], [0, 2], [1, N]],
                )
                dst0 = AP(
                    tensor=xd.tensor,
                    offset=xd.offset,
                    ap=[[CROW, 2 * ssz], [N, 2], [1, N]],
                )
                nc.gpsimd.dma_start(out=dst0, in_=src0)

            # --- identity for PE transpose (needed only ~9us in) ---
            def load_spal():
                if spal_pool:
                    nc.gpsimd.dma_start(out=spal_t[:], in_=spald[:])
                elif spal_split:
                    nc.scalar.dma_start(out=spal_t[:, 0:128], in_=spald[:, 0:128])
                    nc.sync.dma_start(out=spal_t[:, 128:256], in_=spald[:, 128:256])
                else:
                    (nc.sync if sp_only else nc.scalar).dma_start(
                        out=spal_t[:], in_=spald[:]
                    )

            if pal_mm:
                spal_t = cpool.tile([128, 256], spal_dt)
                if not spal_late:
                    load_spal()
                ident = None
            else:
                ident = cpool.tile([128, 128], f32)
                if ident_input:
                    (nc.sync if sp_only else nc.scalar).dma_start(
                        out=ident[:], in_=identd[:]
                    )
                else:
                    ones = cpool.tile([128, 128], f32)
                    nc.vector.memset(ones[:], 1.0)
                    nc.gpsimd.affine_select(
                        out=ident[:],
                        in_=ones[:],
                        pattern=[[1, 128]],
                        compare_op=mybir.AluOpType.is_equal,
                        fill=0.0,
                        base=0,
                        channel_multiplier=-1,
                    )
            if warmup:
                warm = cpool.tile([1, 2], f32)
                nc.vector.memset(warm[:], 1.0)
                nc.scalar.mul(warm[:], warm[:], 1.0)

            # --- hankel tiles: H[16c+p, j*W+u] = x_c[b_j, p+u] ---
            htiles = []
            for s, ssz in enumerate(sgroups):
                ht = hpool.tile([K, ssz * W], f8)
                htiles.append(ht)
                ci = max(i for i in range(len(cgs)) if cstart[i] <= sstart[s])
                assert cstart[ci] + cgs[ci] >= sstart[s] + ssz, "sg spans casts"
                xd = xdups[ci]
                xoff = 2 * (sstart[s] - cstart[ci]) * CROW
                if ssz == 1:
                    src = AP(
                        tensor=xd.tensor,
                        offset=xd.offset + xoff,
                        ap=[[CROW, 2], [1, P], [1, W]],
                    )
                    eng = nc.sync if (sp_only or s % 2 == 0) else nc.scalar
                    eng.dma_start(out=ht[:], in_=src)
                else:
                    for c in range(2):
                        src = AP(
                            tensor=xd.tensor,
                            offset=xd.offset + xoff + c * CROW,
                            ap=[[1, P], [ROW, ssz], [1, W]],
                        )
                        first = (c == 0) != hswap
                        eng = nc.sync if (sp_only or first) else nc.scalar
                        eng.dma_start(out=ht[P * c : P * c + P, :], in_=src)

            if pal_mm and spal_late:
                load_spal()

            rgs = {}

            def finish(g):
                gb = groups[g]
                rows = spool.tile([gb, 256], f32)
                if pal_mm:
                    # one matmul applies transpose+mirror+1/L:
                    # pt2[j,k] = sum_d rg[d,j]*spal[d,k],  spal[d,k]=[d==|k-127|]/L
                    pt2 = tpool.tile([gb, 256], f32)
                    nc.tensor.matmul(pt2[:], rgs[g][:], spal_t[:])
                    nc.vector.tensor_copy(rows[:, 0:255], pt2[:, 0:255])
                else:
                    pt = tpool.tile([gb, 128], f32)
                    nc.tensor.transpose(pt[:], rgs[g][:], ident[:])
                    if act_copy is True or sp_only or (act_copy and g in act_copy):
                        nc.scalar.mul(rows[:, 127:255], pt[:], 1.0)
                    else:
                        nc.vector.tensor_copy(rows[:, 127:255], pt[:])
                    nc.vector.tensor_copy(rows[:, 0:127], pt[:, 127:0:-1])
                src2 = AP(
                    tensor=rows.tensor,
                    offset=rows.offset + 127,
                    ap=[[256, gb], [-1, 128], [1, 128]],
                )
                dst2 = AP(
                    tensor=out,
                    offset=gstart[g] * L * L,
                    ap=[[L * L, gb], [L, 128], [1, 128]],
                )
                if sp_only or g >= ng - last_sp:
                    eng = nc.sync
                else:
                    eng = nc.sync if g % 2 == 0 else nc.scalar
                eng.dma_start(out=dst2, in_=src2)

            for g in range(ng):
                gb = groups[g]
                ps = ppool.tile([128, gb], f32)
                fa = fin_at if fin_at is not None else max(1, gb // 2)
                for j in range(gb):
                    if j == min(fa, gb - 1) and g >= 1:
                        finish(g - 1)
                    b = gstart[g] + j
                    s = b2s[b]
                    ht = htiles[s]
                    col = (b - sstart[s]) * W
                    for tp in range(T // 2):
                        off = col + K * tp
                        lhsT = AP(
                            tensor=ht.tensor,
                            offset=ht.offset + off,
                            ap=[[sgroups[s] * W, K], [P, 2], [1, 128]],
                        )
                        rhs = AP(
                            tensor=ht.tensor,
                            offset=ht.offset + off,
                            ap=[[sgroups[s] * W, K], [P, 2], [1, 1]],
                        )
                        nc.tensor.matmul(
                            ps[:, j : j + 1],
                            lhsT,
                            rhs,
                            start=(tp == 0),
                            stop=(tp == T // 2 - 1),
                            perf_mode=mybir.MatmulPerfMode.DoubleRow,
                        )

                rg = rpool.tile([128, gb], pal_dt if pal_mm else f32)
                if pal_mm:
                    nc.vector.tensor_copy(rg[:], ps[:])
                else:
                    nc.vector.tensor_scalar_mul(rg[:], ps[:], 1.0 / L)
                rgs[g] = rg
            finish(ng - 1)

    nc.compile()
    return nc


def kernel(inputs: np.ndarray) -> np.ndarray:
    global LAST_RESULT
    inputs = np.ascontiguousarray(np.asarray(inputs), dtype=np.float32)
    assert inputs.shape == (B, 2, N), inputs.shape

    if "nc" not in _CACHE:
        _CACHE["nc"] = build_nc()
    nc = _CACHE["nc"]

    k = np.arange(256)
    d = np.arange(128)
    spal = (d[:, None] == np.minimum(np.abs(k[None, :] - 127), 127)).astype(
        np.float32
    ) / L
    spal[:, 255] = 0.0
    in_maps = [
        {"inp": inputs[c * BPC : (c + 1) * BPC], "spald": spal}
        for c in range(NCORES)
    ]
    res = run_bass_kernel_spmd(nc, in_maps, list(range(NCORES)), trace=False)
    LAST_RESULT = res
    outf = np.concatenate([res.results[c]["out"] for c in range(NCORES)], axis=0)
    return outf.reshape(B, L, L, 1).astype(np.float32, copy=False)


# revision 33
# speedup vs baseline: 1.0023x; 1.0009x over previous
"""Trainium2 Bass kernel for nn_CovarianceLayer (Toeplitz-autocorrelation form).

Math: x = inputs[:,0,:] + i*inputs[:,1,:]  (B=256 complex signals, N=1024)
      cov[b,l,m] = Re(hankel @ hankel^H)[l,m] / L  with hankel[b,i,j] = x[b,(j+i)%N]
By circularity cov[b,l,m] = r_b[|l-m|] / L where
      r_b[d] = sum_n ( xr[n]xr[n+d] + xi[n]xi[n+d] )   (indices mod N)
i.e. each [L,L] output tile is a symmetric Toeplitz matrix fully
determined by a 128-lag autocorrelation r_b, so only r_b is computed.

Per-core pipeline (32 batches/core, pure data parallel):
  1. One gpsimd casting DMA per supergroup builds a doubled fp8e4m3 copy
     of x in DRAM (row per batch: [x0 x0 | x1 x1]; the duplication
     realizes the circular wrap). Separate DRAM tiles per supergroup
     keep the dependency tracking fine-grained.
  2. Per 8-batch supergroup, 2 HWDGE DMAs build a packed Hankel tile
     H[16c+p, j*W+u] = x_c[b_j, p+u] (overlapping-window access
     pattern; comps fp8-packed on partition halves).
  3. 32 DoubleRow fp8 matmuls per batch (each contracts K=32 offsets x
     comps times 2 k-tiles) accumulate r_b into one psum column; the
     moving operand is a single column of the same Hankel tile.
  4. Per compute group: DVE drains psum -> SBUF, then ONE f32r matmul
     against a constant palindrome matrix (spal[d,k] = [d==|k-127|]/L)
     applies transpose + mirror + 1/L in one PE op, and a single DVE
     copy stages the palindrome rows s_b[k] = r_b[|k-127|] in SBUF.
  5. One strided DMA per group expands the Toeplitz tiles straight from
     SBUF into the output: out[b,l,m] = s_b[127-l+m] (contiguous 512B
     runs on both sides, ~360GB/s on the DMA engines).
Group sizes taper so the first expansion DMA starts early and the last
group's drain->expand tail is short; expansion DMAs alternate SP/Act
queues and overlap later groups' matmuls.
"""

import numpy as np

import concourse.bacc as bacc
import concourse.mybir as mybir
import concourse.tile as tile
from concourse.bass_types import AP
from concourse.bass_utils import run_bass_kernel_spmd

B, L, N = 256, 128, 1024
NCORES = 8
BPC = B // NCORES  # 32 batches per core

P = 16  # n-offsets per chunk
K = 2 * P  # matmul contraction width
T = N // P  # 64 chunks per batch
W = N - P + 128  # 1136
CROW = 2 * N  # doubled per-comp row in xdup
ROW = 2 * CROW  # 4096 elems per batch

_CACHE = {}
LAST_RESULT = None

SGROUPS = [8, 8, 8, 8]  # hankel supergroup sizes
GROUPS = [6, 6, 6, 6, 4, 2, 2]  # compute/expansion group sizes
CAST_SPLIT = 0  # batches cast in the first (fast) casting DMA; 0 = single


def build_nc(sgroups=None, groups=None, cast_split=CAST_SPLIT, act_copy=False,
             warmup=False, ident_input=True, last_sp=0, sp_only=False,
             pal_mm="f32r", cgroups=None, fin_at=None, pbufs=(4, 4),
             spal_bf16=False, spal_late=True, spal_split=False,
             hswap=False, spal_pool=True, sg0_sp=True):
    f8 = mybir.dt.float8e4
    f32 = mybir.dt.float32
    sgroups = list(SGROUPS if sgroups is None else sgroups)
    groups = list(GROUPS if groups is None else groups)
    assert sum(sgroups) == BPC and sum(groups) == BPC
    sstart = [sum(sgroups[:i]) for i in range(len(sgroups))]
    gstart = [sum(groups[:i]) for i in range(len(groups))]
    ng = len(groups)
    # map batch -> supergroup index
    b2s = []
    for s, sz in enumerate(sgroups):
        b2s += [s] * sz

    nc = bacc.Bacc(
        "TRN2", target_bir_lowering=False, debug=False, num_devices=NCORES
    )
    inp = nc.dram_tensor("inp", [BPC, 2, N], f32, kind="ExternalInput")
    pal_dt = {None: None, "f32": f32, "f32r": mybir.dt.float32r}[pal_mm]
    identd = (
        nc.dram_tensor("identd", [128, 128], f32, kind="ExternalInput")
        if ident_input and not pal_mm
        else None
    )
    spal_dt = {
        "bf16": mybir.dt.bfloat16,
        "f8": mybir.dt.float8e4,
        False: pal_dt,
        True: mybir.dt.bfloat16,
    }[spal_bf16]
    spald = (
        nc.dram_tensor("spald", [128, 256], spal_dt, kind="ExternalInput")
        if pal_mm
        else None
    )
    out = nc.dram_tensor("out", [BPC, L, L], f32, kind="ExternalOutput")

    with tile.TileContext(nc) as tc:
        with (
            tc.tile_pool(name="const", bufs=1) as cpool,
            tc.tile_pool(name="dram", bufs=1, space="DRAM") as dpool,
            tc.tile_pool(name="hank", bufs=len(sgroups)) as hpool,
            tc.tile_pool(name="spal", bufs=ng) as spool,
            tc.tile_pool(name="rr", bufs=ng) as rpool,
            tc.tile_pool(name="psum", bufs=pbufs[0], space="PSUM") as ppool,
            tc.tile_pool(name="pst", bufs=pbufs[1], space="PSUM") as tpool,
        ):
            # --- doubled fp8 signal in DRAM via per-supergroup casting
            # DMAs, emitted FIRST. Each supergroup gets its own DRAM tile so
            # its hankel DMAs depend only on its own cast (tile-granular
            # dependency tracking). ---
            flat = inp[:].rearrange("b c n -> (b c) n")
            cgs = list(cgroups) if cgroups else list(sgroups)
            assert sum(cgs) == BPC
            cstart = [sum(cgs[:i]) for i in range(len(cgs))]
            xdups = []  # one per cast group
            for si, ssz in enumerate(cgs):
                xd = dpool.tile([2 * ssz, CROW], f8)
                xdups.append(xd)
                r0 = 2 * cstart[si]
                src0 = AP(
                    tensor=flat.tensor,
                    offset=flat.offset + r0 * N,
                    ap=[[N, 2 * ssz], [0, 2], [1, N]],
                )
                dst0 = AP(
                    tensor=xd.tensor,
                    offset=xd.offset,
                    ap=[[CROW, 2 * ssz], [N, 2], [1, N]],
                )
                nc.gpsimd.dma_start(out=dst0, in_=src0)

            # --- identity for PE transpose (needed only ~9us in) ---
            def load_spal():
                if spal_pool:
                    nc.gpsimd.dma_start(out=spal_t[:], in_=spald[:])
                elif spal_split:
                    nc.scalar.dma_start(out=spal_t[:, 0:128], in_=spald[:, 0:128])
                    nc.sync.dma_start(out=spal_t[:, 128:256], in_=spald[:, 128:256])
                else:
                    (nc.sync if sp_only else nc.scalar).dma_start(
                        out=spal_t[:], in_=spald[:]
                    )

            if pal_mm:
                spal_t = cpool.tile([128, 256], spal_dt)
                if not spal_late:
                    load_spal()
                ident = None
            else:
                ident = cpool.tile([128, 128], f32)
                if ident_input:
                    (nc.sync if sp_only else nc.scalar).dma_start(
                        out=ident[:], in_=identd[:]
                    )
                else:
                    ones = cpool.tile([128, 128], f32)
                    nc.vector.memset(ones[:], 1.0)
                    nc.gpsimd.affine_select(
                        out=ident[:],
                        in_=ones[:],
                        pattern=[[1, 128]],
                        compare_op=mybir.AluOpType.is_equal,
                        fill=0.0,
                        base=0,
                        channel_multiplier=-1,
                    )
            if warmup:
                warm = cpool.tile([1, 2], f32)
                nc.vector.memset(warm[:], 1.0)
                nc.scalar.mul(warm[:], warm[:], 1.0)

            # --- hankel tiles: H[16c+p, j*W+u] = x_c[b_j, p+u] ---
            htiles = []
            for s, ssz in enumerate(sgroups):
                ht = hpool.tile([K, ssz * W], f8)
                htiles.append(ht)
                ci = max(i for i in range(len(cgs)) if cstart[i] <= sstart[s])
                assert cstart[ci] + cgs[ci] >= sstart[s] + ssz, "sg spans casts"
                xd = xdups[ci]
                xoff = 2 * (sstart[s] - cstart[ci]) * CROW
                if ssz == 1:
                    src = AP(
                        tensor=xd.tensor,
                        offset=xd.offset + xoff,
                        ap=[[CROW, 2], [1, P], [1, W]],
                    )
                    eng = nc.sync if (sp_only or s % 2 == 0) else nc.scalar
                    eng.dma_start(out=ht[:], in_=src)
                else:
                    for c in range(2):
                        src = AP(
                            tensor=xd.tensor,
                            offset=xd.offset + xoff + c * CROW,
                            ap=[[1, P], [ROW, ssz], [1, W]],
                        )
                        first = (c == 0) != hswap
                        if sg0_sp and s == 0:
                            eng = nc.sync
                        else:
                            eng = nc.sync if (sp_only or first) else nc.scalar
                        eng.dma_start(out=ht[P * c : P * c + P, :], in_=src)

            if pal_mm and spal_late:
                load_spal()

            rgs = {}

            def finish(g):
                gb = groups[g]
                rows = spool.tile([gb, 256], f32)
                if pal_mm:
                    # one matmul applies transpose+mirror+1/L:
                    # pt2[j,k] = sum_d rg[d,j]*spal[d,k],  spal[d,k]=[d==|k-127|]/L
                    pt2 = tpool.tile([gb, 256], f32)
                    nc.tensor.matmul(pt2[:], rgs[g][:], spal_t[:])
                    nc.vector.tensor_copy(rows[:, 0:255], pt2[:, 0:255])
                else:
                    pt = tpool.tile([gb, 128], f32)
                    nc.tensor.transpose(pt[:], rgs[g][:], ident[:])
                    if act_copy is True or sp_only or (act_copy and g in act_copy):
                        nc.scalar.mul(rows[:, 127:255], pt[:], 1.0)
                    else:
                        nc.vector.tensor_copy(rows[:, 127:255], pt[:])
                    nc.vector.tensor_copy(rows[:, 0:127], pt[:, 127:0:-1])
                src2 = AP(
                    tensor=rows.tensor,
                    offset=rows.offset + 127,
                    ap=[[256, gb], [-1, 128], [1, 128]],
                )
                dst2 = AP(
                    tensor=out,
                    offset=gstart[g] * L * L,
                    ap=[[L * L, gb], [L, 128], [1, 128]],
                )
                if sp_only or g >= ng - last_sp:
                    eng = nc.sync
                else:
                    eng = nc.sync if g % 2 == 0 else nc.scalar
                eng.dma_start(out=dst2, in_=src2)

            for g in range(ng):
                gb = groups[g]
                ps = ppool.tile([128, gb], f32)
                fa = fin_at if fin_at is not None else max(1, gb // 2)
                for j in range(gb):
                    if j == min(fa, gb - 1) and g >= 1:
                        finish(g - 1)
                    b = gstart[g] + j
                    s = b2s[b]
                    ht = htiles[s]
                    col = (b - sstart[s]) * W
                    for tp in range(T // 2):
                        off = col + K * tp
                        lhsT = AP(
                            tensor=ht.tensor,
                            offset=ht.offset + off,
                            ap=[[sgroups[s] * W, K], [P, 2], [1, 128]],
                        )
                        rhs = AP(
                            tensor=ht.tensor,
                            offset=ht.offset + off,
                            ap=[[sgroups[s] * W, K], [P, 2], [1, 1]],
                        )
                        nc.tensor.matmul(
                            ps[:, j : j + 1],
                            lhsT,
                            rhs,
                            start=(tp == 0),
                            stop=(tp == T // 2 - 1),
                            perf_mode=mybir.MatmulPerfMode.DoubleRow,
                        )

                rg = rpool.tile([128, gb], pal_dt if pal_mm else f32)
                if pal_mm:
                    nc.vector.tensor_copy(rg[:], ps[:])
                else:
                    nc.vector.tensor_scalar_mul(rg[:], ps[:], 1.0 / L)
                rgs[g] = rg
            finish(ng - 1)

    nc.compile()
    return nc


def kernel(inputs: np.ndarray) -> np.ndarray:
    global LAST_RESULT
    inputs = np.ascontiguousarray(np.asarray(inputs), dtype=np.float32)
    assert inputs.shape == (B, 2, N), inputs.shape

    if "nc" not in _CACHE:
        _CACHE["nc"] = build_nc()
    nc = _CACHE["nc"]

    k = np.arange(256)
    d = np.arange(128)
    spal = (d[:, None] == np.minimum(np.abs(k[None, :] - 127), 127)).astype(
        np.float32
    ) / L
    spal[:, 255] = 0.0
    in_maps = [
        {"inp": inputs[c * BPC : (c + 1) * BPC], "spald": spal}
        for c in range(NCORES)
    ]
    res = run_bass_kernel_spmd(nc, in_maps, list(range(NCORES)), trace=False)
    LAST_RESULT = res
    outf = np.concatenate([res.results[c]["out"] for c in range(NCORES)], axis=0)
    return outf.reshape(B, L, L, 1).astype(np.float32, copy=False)


# revision 35
# speedup vs baseline: 1.0042x; 1.0019x over previous
"""Trainium2 Bass kernel for nn_CovarianceLayer (Toeplitz-autocorrelation form).

Math: x = inputs[:,0,:] + i*inputs[:,1,:]  (B=256 complex signals, N=1024)
      cov[b,l,m] = Re(hankel @ hankel^H)[l,m] / L  with hankel[b,i,j] = x[b,(j+i)%N]
By circularity cov[b,l,m] = r_b[|l-m|] / L where
      r_b[d] = sum_n ( xr[n]xr[n+d] + xi[n]xi[n+d] )   (indices mod N)
i.e. each [L,L] output tile is a symmetric Toeplitz matrix fully
determined by a 128-lag autocorrelation r_b, so only r_b is computed.

Per-core pipeline (32 batches/core, pure data parallel):
  1. One gpsimd casting DMA per supergroup builds a doubled fp8e4m3 copy
     of x in DRAM (row per batch: [x0 x0 | x1 x1]; the duplication
     realizes the circular wrap). Separate DRAM tiles per supergroup
     keep the dependency tracking fine-grained.
  2. Per 8-batch supergroup, 2 HWDGE DMAs build a packed Hankel tile
     H[16c+p, j*W+u] = x_c[b_j, p+u] (overlapping-window access
     pattern; comps fp8-packed on partition halves).
  3. 32 DoubleRow fp8 matmuls per batch (each contracts K=32 offsets x
     comps times 2 k-tiles) accumulate r_b into one psum column; the
     moving operand is a single column of the same Hankel tile.
  4. Per compute group: DVE drains psum -> SBUF, then ONE f32r matmul
     against a constant palindrome matrix (spal[d,k] = [d==|k-127|]/L)
     applies transpose + mirror + 1/L in one PE op, and a single DVE
     copy stages the palindrome rows s_b[k] = r_b[|k-127|] in SBUF.
  5. One strided DMA per group expands the Toeplitz tiles straight from
     SBUF into the output: out[b,l,m] = s_b[127-l+m] (contiguous 512B
     runs on both sides, ~360GB/s on the DMA engines).
Group sizes taper so the first expansion DMA starts early and the last
group's drain->expand tail is short; expansion DMAs alternate SP/Act
queues and overlap later groups' matmuls.
"""

import numpy as np

import concourse.bacc as bacc
import concourse.mybir as mybir
import concourse.tile as tile
from concourse.bass_types import AP
from concourse.bass_utils import run_bass_kernel_spmd

B, L, N = 256, 128, 1024
NCORES = 8
BPC = B // NCORES  # 32 batches per core

P = 16  # n-offsets per chunk
K = 2 * P  # matmul contraction width
T = N // P  # 64 chunks per batch
W = N - P + 128  # 1136
CROW = 2 * N  # doubled per-comp row in xdup
ROW = 2 * CROW  # 4096 elems per batch

_CACHE = {}
LAST_RESULT = None

SGROUPS = [8, 8, 8, 8]  # hankel supergroup sizes
GROUPS = [6, 6, 6, 6, 4, 2, 2]  # compute/expansion group sizes
CAST_SPLIT = 0  # batches cast in the first (fast) casting DMA; 0 = single


def build_nc(sgroups=None, groups=None, cast_split=CAST_SPLIT, act_copy=False,
             warmup=False, ident_input=True, last_sp=0, sp_only=False,
             pal_mm="f32r", cgroups=None, fin_at=None, pbufs=(4, 4),
             spal_bf16=False, spal_late=True, spal_split=False,
             hswap=False, spal_pool=True, sg0_sp=2):
    f8 = mybir.dt.float8e4
    f32 = mybir.dt.float32
    sgroups = list(SGROUPS if sgroups is None else sgroups)
    groups = list(GROUPS if groups is None else groups)
    assert sum(sgroups) == BPC and sum(groups) == BPC
    sstart = [sum(sgroups[:i]) for i in range(len(sgroups))]
    gstart = [sum(groups[:i]) for i in range(len(groups))]
    ng = len(groups)
    # map batch -> supergroup index
    b2s = []
    for s, sz in enumerate(sgroups):
        b2s += [s] * sz

    nc = bacc.Bacc(
        "TRN2", target_bir_lowering=False, debug=False, num_devices=NCORES
    )
    inp = nc.dram_tensor("inp", [BPC, 2, N], f32, kind="ExternalInput")
    pal_dt = {None: None, "f32": f32, "f32r": mybir.dt.float32r}[pal_mm]
    identd = (
        nc.dram_tensor("identd", [128, 128], f32, kind="ExternalInput")
        if ident_input and not pal_mm
        else None
    )
    spal_dt = {
        "bf16": mybir.dt.bfloat16,
        "f8": mybir.dt.float8e4,
        False: pal_dt,
        True: mybir.dt.bfloat16,
    }[spal_bf16]
    spald = (
        nc.dram_tensor("spald", [128, 256], spal_dt, kind="ExternalInput")
        if pal_mm
        else None
    )
    out = nc.dram_tensor("out", [BPC, L, L], f32, kind="ExternalOutput")

    with tile.TileContext(nc) as tc:
        with (
            tc.tile_pool(name="const", bufs=1) as cpool,
            tc.tile_pool(name="dram", bufs=1, space="DRAM") as dpool,
            tc.tile_pool(name="hank", bufs=len(sgroups)) as hpool,
            tc.tile_pool(name="spal", bufs=ng) as spool,
            tc.tile_pool(name="rr", bufs=ng) as rpool,
            tc.tile_pool(name="psum", bufs=pbufs[0], space="PSUM") as ppool,
            tc.tile_pool(name="pst", bufs=pbufs[1], space="PSUM") as tpool,
        ):
            # --- doubled fp8 signal in DRAM via per-supergroup casting
            # DMAs, emitted FIRST. Each supergroup gets its own DRAM tile so
            # its hankel DMAs depend only on its own cast (tile-granular
            # dependency tracking). ---
            flat = inp[:].rearrange("b c n -> (b c) n")
            cgs = list(cgroups) if cgroups else list(sgroups)
            assert sum(cgs) == BPC
            cstart = [sum(cgs[:i]) for i in range(len(cgs))]
            xdups = []  # one per cast group
            for si, ssz in enumerate(cgs):
                xd = dpool.tile([2 * ssz, CROW], f8)
                xdups.append(xd)
                r0 = 2 * cstart[si]
                src0 = AP(
                    tensor=flat.tensor,
                    offset=flat.offset + r0 * N,
                    ap=[[N, 2 * ssz], [0, 2], [1, N]],
                )
                dst0 = AP(
                    tensor=xd.tensor,
                    offset=xd.offset,
                    ap=[[CROW, 2 * ssz], [N, 2], [1, N]],
                )
                nc.gpsimd.dma_start(out=dst0, in_=src0)

            # --- identity for PE transpose (needed only ~9us in) ---
            def load_spal():
                if spal_pool:
                    nc.gpsimd.dma_start(out=spal_t[:], in_=spald[:])
                elif spal_split:
                    nc.scalar.dma_start(out=spal_t[:, 0:128], in_=spald[:, 0:128])
                    nc.sync.dma_start(out=spal_t[:, 128:256], in_=spald[:, 128:256])
                else:
                    (nc.sync if sp_only else nc.scalar).dma_start(
                        out=spal_t[:], in_=spald[:]
                    )

            if pal_mm:
                spal_t = cpool.tile([128, 256], spal_dt)
                if not spal_late:
                    load_spal()
                ident = None
            else:
                ident = cpool.tile([128, 128], f32)
                if ident_input:
                    (nc.sync if sp_only else nc.scalar).dma_start(
                        out=ident[:], in_=identd[:]
                    )
                else:
                    ones = cpool.tile([128, 128], f32)
                    nc.vector.memset(ones[:], 1.0)
                    nc.gpsimd.affine_select(
                        out=ident[:],
                        in_=ones[:],
                        pattern=[[1, 128]],
                        compare_op=mybir.AluOpType.is_equal,
                        fill=0.0,
                        base=0,
                        channel_multiplier=-1,
                    )
            if warmup:
                warm = cpool.tile([1, 2], f32)
                nc.vector.memset(warm[:], 1.0)
                nc.scalar.mul(warm[:], warm[:], 1.0)

            # --- hankel tiles: H[16c+p, j*W+u] = x_c[b_j, p+u] ---
            htiles = []
            for s, ssz in enumerate(sgroups):
                ht = hpool.tile([K, ssz * W], f8)
                htiles.append(ht)
                ci = max(i for i in range(len(cgs)) if cstart[i] <= sstart[s])
                assert cstart[ci] + cgs[ci] >= sstart[s] + ssz, "sg spans casts"
                xd = xdups[ci]
                xoff = 2 * (sstart[s] - cstart[ci]) * CROW
                if ssz == 1:
                    src = AP(
                        tensor=xd.tensor,
                        offset=xd.offset + xoff,
                        ap=[[CROW, 2], [1, P], [1, W]],
                    )
                    eng = nc.sync if (sp_only or s % 2 == 0) else nc.scalar
                    eng.dma_start(out=ht[:], in_=src)
                else:
                    for c in range(2):
                        src = AP(
                            tensor=xd.tensor,
                            offset=xd.offset + xoff + c * CROW,
                            ap=[[1, P], [ROW, ssz], [1, W]],
                        )
                        first = (c == 0) != hswap
                        if sg0_sp is not False and s < int(sg0_sp):
                            eng = nc.sync
                        else:
                            eng = nc.sync if (sp_only or first) else nc.scalar
                        eng.dma_start(out=ht[P * c : P * c + P, :], in_=src)

            if pal_mm and spal_late:
                load_spal()

            rgs = {}

            def finish(g):
                gb = groups[g]
                rows = spool.tile([gb, 256], f32)
                if pal_mm:
                    # one matmul applies transpose+mirror+1/L:
                    # pt2[j,k] = sum_d rg[d,j]*spal[d,k],  spal[d,k]=[d==|k-127|]/L
                    pt2 = tpool.tile([gb, 256], f32)
                    nc.tensor.matmul(pt2[:], rgs[g][:], spal_t[:])
                    nc.vector.tensor_copy(rows[:, 0:255], pt2[:, 0:255])
                else:
                    pt = tpool.tile([gb, 128], f32)
                    nc.tensor.transpose(pt[:], rgs[g][:], ident[:])
                    if act_copy is True or sp_only or (act_copy and g in act_copy):
                        nc.scalar.mul(rows[:, 127:255], pt[:], 1.0)
                    else:
                        nc.vector.tensor_copy(rows[:, 127:255], pt[:])
                    nc.vector.tensor_copy(rows[:, 0:127], pt[:, 127:0:-1])
                src2 = AP(
                    tensor=rows.tensor,
                    offset=rows.offset + 127,
                    ap=[[256, gb], [-1, 128], [1, 128]],
                )
                dst2 = AP(
                    tensor=out,
                    offset=gstart[g] * L * L,
                    ap=[[L * L, gb], [L, 128], [1, 128]],
                )
                if sp_only or g >= ng - last_sp:
                    eng = nc.sync
                else:
                    eng = nc.sync if g % 2 == 0 else nc.scalar
                eng.dma_start(out=dst2, in_=src2)

            for g in range(ng):
                gb = groups[g]
                ps = ppool.tile([128, gb], f32)
                fa = fin_at if fin_at is not None else max(1, gb // 2)
                for j in range(gb):
                    if j == min(fa, gb - 1) and g >= 1:
                        finish(g - 1)
                    b = gstart[g] + j
                    s = b2s[b]
                    ht = htiles[s]
                    col = (b - sstart[s]) * W
                    for tp in range(T // 2):
                        off = col + K * tp
                        lhsT = AP(
                            tensor=ht.tensor,
                            offset=ht.offset + off,
                            ap=[[sgroups[s] * W, K], [P, 2], [1, 128]],
                        )
                        rhs = AP(
                            tensor=ht.tensor,
                            offset=ht.offset + off,
                            ap=[[sgroups[s] * W, K], [P, 2], [1, 1]],
                        )
                        nc.tensor.matmul(
                            ps[:, j : j + 1],
                            lhsT,
                            rhs,
                            start=(tp == 0),
                            stop=(tp == T // 2 - 1),
                            perf_mode=mybir.MatmulPerfMode.DoubleRow,
                        )

                rg = rpool.tile([128, gb], pal_dt if pal_mm else f32)
                if pal_mm:
                    nc.vector.tensor_copy(rg[:], ps[:])
                else:
                    nc.vector.tensor_scalar_mul(rg[:], ps[:], 1.0 / L)
                rgs[g] = rg
            finish(ng - 1)

    nc.compile()
    return nc


def kernel(inputs: np.ndarray) -> np.ndarray:
    global LAST_RESULT
    inputs = np.ascontiguousarray(np.asarray(inputs), dtype=np.float32)
    assert inputs.shape == (B, 2, N), inputs.shape

    if "nc" not in _CACHE:
        _CACHE["nc"] = build_nc()
    nc = _CACHE["nc"]

    k = np.arange(256)
    d = np.arange(128)
    spal = (d[:, None] == np.minimum(np.abs(k[None, :] - 127), 127)).astype(
        np.float32
    ) / L
    spal[:, 255] = 0.0
    in_maps = [
        {"inp": inputs[c * BPC : (c + 1) * BPC], "spald": spal}
        for c in range(NCORES)
    ]
    res = run_bass_kernel_spmd(nc, in_maps, list(range(NCORES)), trace=False)
    LAST_RESULT = res
    outf = np.concatenate([res.results[c]["out"] for c in range(NCORES)], axis=0)
    return outf.reshape(B, L, L, 1).astype(np.float32, copy=False)
